# revision 6
# baseline (speedup 1.0000x reference)
"""CondConv2d (moe_routing) Trainium2 Bass kernel.

Full-input contract: kernel(**inputs) takes the unsharded inputs
  x      (32, 256, 56, 56) f32
  weight (2048, 256, 3, 3) f32   -- expert bank, (E*COUT, CIN, 3, 3), E=8
  fc_w   (8, 256) f32
  fc_b   (8,) f32
and returns the full (32, 256, 56, 56) f32 output of

  gate = sigmoid(mean_hw(x) @ fc_w.T + fc_b)              # (n, 8)
  w    = (gate @ weight.reshape(8, -1)).reshape(n, 256, 256, 3, 3)
  out[s] = conv2d(x[s], w[s], padding=1)

Sharding: data-parallel over batch across 8 NeuronCores (4 samples/core),
expert bank + fc params replicated.

Per-core program (heavy matmuls in float32r = full PE rate):
  phase A  gating: x loaded flat, reduce_sum over H*W, tiny PE matmul with
           fc_w^T, sigmoid on ACT (scale=1/3136 folds the mean), gates
           broadcast across partitions via a DRAM roundtrip.
  phase B  expert mixing ON the PE with an expert-interleaved contraction:
           bank rows are loaded as K-tiles whose 128 partitions are
           (e, j) = 8 experts x 16 output-channel lanes, so ONE matmul per
           16-channel group both sums over all 8 experts and transposes the
           bank into the (ci, co) lhsT layout the conv needs:
             out[ci, (s, co')] = sum_(e,j) bank[(e,co),ci] * R[(e,j),(s,c')]
             R[(e,j), (s,c')]  = g[s,e] * delta(j, c')       (N=64 matmuls)
  phase C  3x3 conv as 18 accumulating matmuls (2 ci tiles x 9 taps) per
           448-column PSUM chunk (8 output rows) over a zero-padded 58-wide
           image layout in SBUF.
"""

import numpy as np

import concourse.bass as bass
import concourse.mybir as mybir
import concourse.tile as tile
from concourse import bacc
from concourse.bass_utils import run_bass_kernel_spmd
from concourse.masks import make_identity

# Problem constants (hardcoded per contract).
N_FULL = 32
NCORES = 8
NS = N_FULL // NCORES  # 4 samples per core
E = 8
CIN = 256
COUT = 256
K = 3
H = W = 56
PW = W + 2  # padded row width 58
HW = H * W  # 3136
CH = 8 * W  # 448 output columns per PSUM chunk (8 rows x 56 cols)
NCHUNK = H // 8  # 7
FPAD = PW * (H + 2)  # 3364 padded-x free size
CIT = CIN // 128  # 2 contraction tiles
COT = COUT // 128  # 2 output-partition tiles

f32 = mybir.dt.float32
f32r = mybir.dt.float32r

_CACHED_NC = None


def _build(repeat: int = 1):
    nc = bacc.Bacc(trn_type="TRN2", target_bir_lowering=False, debug=False)

    x_d = nc.dram_tensor("x", (NS, CIN, H, W), f32, kind="ExternalInput").ap()
    w_d = nc.dram_tensor(
        "weight", (E * COUT, CIN, K, K), f32, kind="ExternalInput"
    ).ap()
    fcw_d = nc.dram_tensor("fc_w", (E, CIN), f32, kind="ExternalInput").ap()
    fcb_d = nc.dram_tensor("fc_b", (E,), f32, kind="ExternalInput").ap()
    out_d = nc.dram_tensor("out", (NS, COUT, H, W), f32, kind="ExternalOutput").ap()

    # bank viewed as (e, co, ci*9): row (e,co), 2304 wide
    bank = w_d.rearrange("r c h w -> r (c h w)")  # (2048, 2304)
    # tiled 16x16 identity constant: tI[p, c] = 1 iff p % 16 == c
    tI_d = nc.inline_tensor(
        np.tile(np.eye(16, dtype=np.float32), (8, 1)), name="tiled_eye16"
    ).ap()

    with tile.TileContext(nc) as tc:
      for _rep in range(repeat):
        with (
            tc.tile_pool(name="smalls", bufs=1) as smalls,
            tc.tile_pool(name="wmix", bufs=COT * CIT * 9) as wmpool,
            tc.tile_pool(name="dram", bufs=1, space="DRAM") as dramp,
            tc.tile_pool(name="ps", bufs=8, space="PSUM") as pp,
        ):
            # ---------------- phase A: gating ----------------
            means = []
            for ci_t in range(CIT):
                means.append(
                    smalls.tile([128, NS], f32, tag=f"means{ci_t}", name=f"means{ci_t}")
                )
            fcwt = []
            for ci_t in range(CIT):
                t = smalls.tile([128, E], f32, tag=f"fcwt{ci_t}", name=f"fcwt{ci_t}")
                # fc_w^T slice: (ci in tile, e) -- tiny strided DMA
                nc.sync.dma_start(
                    out=t,
                    in_=fcw_d.rearrange("e c -> c e")[
                        ci_t * 128 : (ci_t + 1) * 128, :
                    ],
                )
                fcwt.append(t)
            fcb_sb = smalls.tile([E, 1], f32, tag="fcb")
            nc.sync.dma_start(out=fcb_sb, in_=fcb_d.unsqueeze(1))
            tI_sb = smalls.tile([128, 16], f32, tag="tI")
            nc.sync.dma_start(out=tI_sb, in_=tI_d)

            with tc.tile_pool(name="xflat", bufs=2) as xfp:
                for s in range(NS):
                    for ci_t in range(CIT):
                        xt = xfp.tile([128, HW], f32, tag="xflat", name="xflat")
                        nc.sync.dma_start(
                            out=xt,
                            in_=x_d[s, ci_t * 128 : (ci_t + 1) * 128].rearrange(
                                "c h w -> c (h w)"
                            ),
                        )
                        nc.vector.reduce_sum(
                            out=means[ci_t][:, s : s + 1],
                            in_=xt[:],
                            axis=mybir.AxisListType.X,
                        )

            # logits[e, s] = sum_ci fc_w[e, ci] * xsum[ci, s]
            ps_g = pp.tile([E, NS], f32, tag="ps", name="ps_g")
            for ci_t in range(CIT):
                nc.tensor.matmul(
                    ps_g[:],
                    fcwt[ci_t][:],
                    means[ci_t][:],
                    start=(ci_t == 0),
                    stop=(ci_t == CIT - 1),
                )
            gate_sb = smalls.tile([E, NS], f32, tag="gate")
            # gate = sigmoid(logits / (H*W) + fc_b)
            nc.scalar.activation(
                gate_sb[:],
                ps_g[:],
                mybir.ActivationFunctionType.Sigmoid,
                bias=fcb_sb[:],
                scale=1.0 / float(HW),
            )
            # gate roundtrip: gv_s[p] = g[s, p//16]  (per-partition scalars)
            gdram = dramp.tile([E, NS], f32, tag="gd", name="gdram")
            nc.sync.dma_start(out=gdram, in_=gate_sb)
            gvs = []
            for s in range(NS):
                gv = smalls.tile([128, 1], f32, tag=f"gv{s}", name=f"gv{s}")
                src = bass.AP(
                    tensor=gdram.tensor,
                    offset=gdram.offset + s,
                    ap=[[NS, E], [0, 16]],
                )
                nc.sync.dma_start(out=gv[:], in_=src)
                gvs.append(gv)

            # R[(e,j), (s,c')] = g[s,e] * delta(j, c')
            rt = smalls.tile([128, NS * 16], f32r, tag="rt")
            for s in range(NS):
                nc.vector.tensor_scalar_mul(
                    rt[:, s * 16 : (s + 1) * 16], tI_sb[:], gvs[s][:]
                )

            # ---------------- phase B: expert mixing on PE ----------------
            # wm[(co_t, ci_t, tap)][ci_l, s*128 + co_l] =
            #     sum_e g[s,e] * weight[e, co_t*128+co_l, ci_t*128+ci_l, tap]
            wm = {}
            with tc.tile_pool(name="bank", bufs=10) as bkp:
                for co_t in range(COT):
                    bts = []
                    for cbl in range(8):
                        # K-tile rows: (e, j) -> bank row e*256 + co_t*128
                        #                         + cbl*16 + j
                        bt = bkp.tile([128, CIT, 128, 9], f32r, tag="bank", name="bank")
                        src = bass.AP(
                            tensor=bank.tensor,
                            offset=bank.offset
                            + (co_t * 128 + cbl * 16) * 2304,
                            ap=[[256 * 2304, E], [2304, 16], [1, 2304]],
                        )
                        nc.gpsimd.dma_start(
                            out=bt[:].rearrange("p a b c -> p (a b c)"),
                            in_=src,
                        )
                        bts.append(bt)
                    for ci_t in range(CIT):
                        for tap in range(9):
                            ps_m = pp.tile(
                                [128, NS, 128], f32, tag="ps", name="ps_m"
                            )
                            for cbl in range(8):
                                nc.tensor.matmul(
                                    ps_m[:, :, cbl * 16 : (cbl + 1) * 16],
                                    bts[cbl][:, ci_t, :, tap],
                                    rt[:],
                                    start=True,
                                    stop=True,
                                )
                            wt = wmpool.tile([128, NS * 128], f32r, tag="wm", name="wm")
                            nc.scalar.copy(
                                wt[:], ps_m[:].rearrange("p a b -> p (a b)")
                            )
                            wm[(co_t, ci_t, tap)] = wt

            # ---------------- phase C: conv ----------------
            with (
                tc.tile_pool(name="xpad", bufs=2 * CIT) as xpp,
                tc.tile_pool(name="outp", bufs=2) as op,
            ):
                for s in range(NS):
                    xts = []
                    for ci_t in range(CIT):
                        xt = xpp.tile([128, FPAD], f32r, tag="xpad", name="xpad")
                        xf = xt[:].bitcast(f32)
                        # zero only the halo: top row, bottom row, and the
                        # (col 57, col 0) pairs between consecutive rows
                        nc.vector.memset(xf[:, 0:PW], 0.0)
                        nc.vector.memset(xf[:, FPAD - PW : FPAD], 0.0)
                        pairs = xf[:, PW - 1 : PW - 1 + (H + 1) * PW].rearrange(
                            "p (a b) -> p a b", b=PW
                        )[:, :, 0:2]
                        nc.vector.memset(pairs, 0.0)
                        nc.gpsimd.dma_start(
                            out=xt[:]
                            .rearrange("p (h w) -> p h w", w=PW)[
                                :, 1 : H + 1, 1 : W + 1
                            ],
                            in_=x_d[s, ci_t * 128 : (ci_t + 1) * 128],
                        )
                        xts.append(xt)
                    for co_t in range(COT):
                        ot = op.tile([128, HW], f32, tag="outp", name="outp")
                        for c in range(NCHUNK):
                            ps_c = pp.tile([128, CH], f32, tag="ps", name="ps_c")
                            i = 0
                            for ci_t in range(CIT):
                                xv = xts[ci_t][:].rearrange("p (h w) -> p h w", w=PW)
                                for kh in range(K):
                                    for kw in range(K):
                                        tap = kh * K + kw
                                        rhs = xv[
                                            :, 8 * c + kh : 8 * c + kh + 8, kw : kw + W
                                        ]
                                        nc.tensor.matmul(
                                            ps_c[:],
                                            wm[(co_t, ci_t, tap)][
                                                :, s * 128 : (s + 1) * 128
                                            ],
                                            rhs,
                                            start=(i == 0),
                                            stop=(i == CIT * 9 - 1),
                                        )
                                        i += 1
                            nc.scalar.copy(ot[:, c * CH : (c + 1) * CH], ps_c[:])
                        nc.sync.dma_start(
                            out=out_d[s, co_t * 128 : (co_t + 1) * 128],
                            in_=ot[:].rearrange("p (h w) -> p h w", w=W),
                        )

    nc.compile()
    return nc


def _get_nc():
    global _CACHED_NC
    if _CACHED_NC is None:
        _CACHED_NC = _build()
    return _CACHED_NC


def kernel(x, weight, fc_w, fc_b):
    assert x.shape == (N_FULL, CIN, H, W), x.shape
    assert weight.shape == (E * COUT, CIN, K, K), weight.shape
    x = np.ascontiguousarray(x, dtype=np.float32)
    weight = np.ascontiguousarray(weight, dtype=np.float32)
    fc_w = np.ascontiguousarray(fc_w, dtype=np.float32)
    fc_b = np.ascontiguousarray(fc_b, dtype=np.float32)

    nc = _get_nc()
    in_maps = [
        {
            "x": np.ascontiguousarray(x[i * NS : (i + 1) * NS]),
            "weight": weight,
            "fc_w": fc_w,
            "fc_b": fc_b,
        }
        for i in range(NCORES)
    ]
    res = run_bass_kernel_spmd(nc, in_maps, core_ids=list(range(NCORES)))
    out = np.concatenate([res.results[i]["out"] for i in range(NCORES)], axis=0)
    return out
